# revision 5
# baseline (speedup 1.0000x reference)
"""Trainium2 Bass kernel for nn_KuramotoChamber (Kuramoto oscillator chamber).

reference:
    theta = phase[:, None] * omega[None, :]           # (B, 6)
    3x:  dtheta_i = sum_j K[i,j]*sin(theta_j - theta_i); theta += 0.1*dtheta
    out = sin(theta) @ W.T                            # (B, 512)

B = 262144, output is 512 MB fp32 -> memory (output-write) bound; the
per-core roofline is the 64 MiB output write at ~360 GB/s = ~187 us.
Sharding: pure data parallel over the batch across 8 cores.

Final design (TimelineSim ~211 us/core vs 269 us baseline; HW-verified
rel err 2.3e-3):
  - matmuls in bf16 (1 PE cycle/row vs 4 for fp32); bf16 spad/transposes.
  - PSUM->SBUF output copies on ACT (pairs < half) and Pool (rest), off
    the DVE chain; ACT sins never queue behind bulk copies.
  - one output DMA per copy engine per macro (single-wait DMAs).
  - macros emitted pairwise-interleaved (2-deep software pipeline) to
    hide ACT sin latency inside the DVE chain.
  - macro sizes ramp 4,4,8,16... so the first output DMA fires after a
    ~9 us mini-chain instead of a 24 us full-macro chain (DMA engines
    are gap-free in steady state; the fill was the remaining loss).

Per-core dataflow (BC = 32768 batch rows = 128 partitions x 256 groups):
  - consolidated const DMA: [phase (128,256) | wtrep f32 | 0.1K | omega |
    identity f32]; wtrep/identity converted to bf16 on-chip once.
  - chain per macro (DVE): theta init; 3x [diff sub, ACT Sin, *K mul,
    segment-reduce, add]; final sin -> bf16 spad; PE transposes -> psT;
    DVE copy -> sT; matmuls (K=6 strips at partitions {0,32,64,96},
    N=512) -> PSUM f32; ACT/Pool copies -> outsb f32; DMAs out.
"""

import os

import numpy as np

B = 262144
N_CORES = 8
BC = B // N_CORES  # 32768 per core
E = 512
N = 6
NN = N * N  # 36
P = 128
G = BC // P  # 256 groups per core

# macro-size ramp (groups per macro); sum must be G
MSIZES = [4] + [8] * 10 + [12] + [16] * 10
assert sum(MSIZES) == G
SOLO_COUNT = 1  # first macros emitted un-interleaved (latency over throughput)
PE_WARMUP = 0   # dummy transposes before first matmul (PE p-state ramp)
FINE_DMA_COUNT = 9  # macros using one DMA per pair (fill phase)
WEAVE_GROUPS = 0  # emission groups prefixed with PE warmup transposes
WEAVE_N = 0
SUB_ON_POOL = False  # diff-sub on GPSIMD instead of DVE
MUL_ON_POOL = True   # K-mult on GPSIMD instead of DVE
RED_ON_POOL = False  # segment-reduce on GPSIMD instead of DVE
ACT_COPY_PAIRS = 6   # pairs copied by ACT; the rest by DVE (Pool cannot
                     # read PSUM on this hardware)
COPY_PRIO = 1_000_000  # (unused) legacy absolute copy priority
COPY_LAG = 80  # copies scheduled as-if emitted this many instructions later
FILL_MACROS = 2   # macros before steady-state engine assignment kicks in
FILL_ACT_PAIRS = 6  # copy split during fill (DVE has slack there)
CHAIN_WARM_MACROS = 3  # macros whose init is followed by a PE warmup burst
SPLIT_COPY_MACROS = 0  # macros with per-single-tp copies+DMAs (fill latency)
CHAIN_WARM_N = 32      # transposes per burst (keeps PE p-state hot to MM)
WORK_BUFS = 6
BIG_BUFS = 6
OUTSB_BUFS = 4

# consolidated const layout: [phase(256) | wt(512) | krep(36) | om(6) | id(128)]
OFF_PHASE = 0
OFF_WT = OFF_PHASE + G
OFF_K = OFF_WT + E
OFF_OM = OFF_K + NN
OFF_ID = OFF_OM + N
CIN_W = OFF_ID + P  # 938


def build_bass():
    import concourse.bass as bass
    import concourse.mybir as mybir
    import concourse.tile as tile

    f32 = mybir.dt.float32
    bf16 = mybir.dt.bfloat16
    Sin = mybir.ActivationFunctionType.Sin
    sub_op = mybir.AluOpType.subtract
    mult_op = mybir.AluOpType.mult

    nc = bass.Bass()
    cin = nc.dram_tensor("cin", [P, CIN_W], f32, kind="ExternalInput")
    out = nc.dram_tensor("out", [BC, E], f32, kind="ExternalOutput")

    # per-macro start group offsets
    starts = []
    s = 0
    for ms in MSIZES:
        starts.append(s)
        s += ms

    with tile.TileContext(nc) as tc:
        with (
            tc.tile_pool(name="consts", bufs=1) as consts,
            tc.tile_pool(name="work", bufs=WORK_BUFS) as work,
            tc.tile_pool(name="big", bufs=BIG_BUFS) as big,
            tc.tile_pool(name="outsb", bufs=OUTSB_BUFS) as outsb_pool,
            tc.tile_pool(name="pst", bufs=2, space="PSUM") as pst_pool,
            tc.tile_pool(name="outps", bufs=3, space="PSUM") as outps_pool,
        ):
            cin_sb = consts.tile([P, CIN_W], f32)
            # chain-critical constants first (phase/K/omega), then wt/id;
            # the chain's first op waits only on the first, smaller DMA.
            nc.sync.dma_start(out=cin_sb[:, OFF_PHASE:OFF_WT], in_=cin[:, OFF_PHASE:OFF_WT])
            nc.sync.dma_start(out=cin_sb[:, OFF_K:CIN_W], in_=cin[:, OFF_K:CIN_W])
            nc.sync.dma_start(out=cin_sb[:, OFF_WT:OFF_K], in_=cin[:, OFF_WT:OFF_K])
            phase_sb = cin_sb[:, OFF_PHASE:OFF_WT]
            wt_sb = cin_sb[:, OFF_WT:OFF_K]
            krep_sb = cin_sb[:, OFF_K:OFF_OM]
            om_sb = cin_sb[:, OFF_OM:OFF_ID]
            id_sb = cin_sb[:, OFF_ID:CIN_W]

            # One-time bf16 conversions (each also serves as that engine's
            # first-touch wait on the const DMA).
            wt_bf = consts.tile([P, E], bf16)
            nc.scalar.copy(out=wt_bf, in_=wt_sb)
            id_bf = consts.tile([P, P], bf16)
            nc.gpsimd.tensor_copy(out=id_bf, in_=id_sb)

            # Persistent bf16 sin staging tiles (ping-pong across macros);
            # memset once so pad columns flowing through the transpose hold
            # defined values.
            spads = [
                consts.tile([P, 4 * P], bf16, name=f"spad{i}", tag=f"spad{i}")
                for i in range(2)
            ]
            for sp in spads:
                nc.vector.memset(sp, 0.0)

            # PE p-state warmup: a stream of dummy transposes ramps the PE
            # clock before the first real matmuls (cold-state matmuls cost
            # 2-4x); results are discarded (psT ring slots get overwritten).
            for _ in range(PE_WARMUP):
                psw = pst_pool.tile([P, P], bf16, name="psw", tag="psT")
                nc.tensor.transpose(out=psw[:, 0:P], in_=id_bf, identity=id_bf)

            # DRAM view: row b = p*256 + g  (g = absolute group index)
            out3 = out[:, :].rearrange("(p g) e -> p g e", p=P, g=G)

            kview = krep_sb.rearrange("p (i j) -> p i j", j=N).unsqueeze(1)
            omview = om_sb.unsqueeze(1)

            def emit_init(ctx, m):
                ms = MSIZES[m]
                ctx["outsb"][m] = outsb_pool.tile(
                    [P, 16 * E], f32, name=f"outsb{m}", tag="outsb"
                )
                ctx["theta"][m] = work.tile(
                    [P, 16 * N], f32, name=f"theta{m}", tag="theta"
                )
                th3 = ctx["theta"][m][:, : ms * N].rearrange(
                    "p (t n) -> p t n", n=N
                )
                ctx["th3"][m] = th3
                g0 = starts[m]
                ph = phase_sb[:, g0 : g0 + ms]
                nc.vector.tensor_tensor(
                    out=th3,
                    in0=ph.unsqueeze(2).broadcast_to([P, ms, N]),
                    in1=omview.broadcast_to([P, ms, N]),
                    op=mult_op,
                )
                if m < CHAIN_WARM_MACROS:
                    emit_warmup(CHAIN_WARM_N)

            def emit_sub(ctx, m, r):
                ms = MSIZES[m]
                th3 = ctx["th3"][m]
                diff = big.tile([P, 16 * NN], f32, name=f"diff{m}", tag="diff")
                ctx["diff"][m] = diff
                eng = nc.gpsimd if SUB_ON_POOL else nc.vector
                eng.tensor_tensor(
                    out=diff[:, : ms * NN].rearrange(
                        "p (t i j) -> p t i j", i=N, j=N
                    ),
                    in0=th3.unsqueeze(2).broadcast_to([P, ms, N, N]),
                    in1=th3.unsqueeze(3).broadcast_to([P, ms, N, N]),
                    op=sub_op,
                )

            def emit_sin(ctx, m, r):
                ms = MSIZES[m]
                sdiff = big.tile([P, 16 * NN], f32, name=f"sdiff{m}", tag="sdiff")
                ctx["sdiff"][m] = sdiff
                nc.scalar.activation(
                    out=sdiff[:, : ms * NN],
                    in_=ctx["diff"][m][:, : ms * NN],
                    func=Sin,
                )

            def emit_mulredadd(ctx, m, r):
                ms = MSIZES[m]
                prod = big.tile([P, 16 * NN], f32, name=f"prod{m}", tag="prod")
                mul_pool = MUL_ON_POOL and m >= FILL_MACROS
                meng = nc.gpsimd if mul_pool else nc.vector
                meng.tensor_tensor(
                    out=prod[:, : ms * NN].rearrange(
                        "p (t i j) -> p t i j", i=N, j=N
                    ),
                    in0=ctx["sdiff"][m][:, : ms * NN].rearrange(
                        "p (t i j) -> p t i j", i=N, j=N
                    ),
                    in1=kview.broadcast_to([P, ms, N, N]),
                    op=mult_op,
                )
                dth = work.tile([P, 16 * N], f32, name=f"dth{m}", tag="dth")
                reng = nc.gpsimd if RED_ON_POOL else nc.vector
                reng.reduce_sum(
                    out=dth[:, : ms * N],
                    in_=prod[:, : ms * NN].rearrange("p (ti j) -> p ti j", j=N),
                    axis=mybir.AxisListType.X,
                )
                nc.vector.tensor_add(
                    out=ctx["theta"][m][:, : ms * N],
                    in0=ctx["theta"][m][:, : ms * N],
                    in1=dth[:, : ms * N],
                )

            def emit_fsin(ctx, m):
                ms = MSIZES[m]
                qb = (ms + 3) // 4  # q-blocks of up to 4 groups
                rdim = min(ms, 4)
                spad = spads[m % 2]
                ctx["spad"][m] = spad
                # group t = 4q+r at cols [128q+32r, +6)
                sp4 = spad[:, : qb * P].rearrange(
                    "p (q r c) -> p q r c", q=qb, r=rdim
                )
                nc.scalar.activation(
                    out=sp4[:, :, :, 0:N],
                    in_=ctx["theta"][m][:, : ms * N].rearrange(
                        "p (q r n) -> p q r n", q=qb, r=rdim
                    ),
                    func=Sin,
                )

            def emit_transpose(ctx, m):
                ms = MSIZES[m]
                qb = (ms + 3) // 4
                spad = ctx["spad"][m]
                psT = pst_pool.tile([P, 4 * P], bf16, name=f"psT{m}", tag="psT")
                for q in range(qb):
                    nc.tensor.transpose(
                        out=psT[:, q * P : (q + 1) * P],
                        in_=spad[:, q * P : (q + 1) * P],
                        identity=id_bf,
                    )
                sT = work.tile([P, 4 * P], bf16, name=f"sT{m}", tag="sT")
                ctx["sT"][m] = sT
                nc.vector.tensor_copy(
                    out=sT[:, : qb * P], in_=psT[:, : qb * P]
                )

            def emit_matmuls(ctx, m):
                ms = MSIZES[m]
                sT = ctx["sT"][m]
                outsb = ctx["outsb"][m]
                npairs = ms // 2
                acp = ACT_COPY_PAIRS if m >= FILL_MACROS else FILL_ACT_PAIRS
                act_pairs = max(1, (npairs * acp) // 8)
                g0 = starts[m]
                outsb3 = outsb[:, : ms * E].rearrange(
                    "p (t e) -> p t e", t=ms
                )
                for pr in range(npairs):
                    ops = outps_pool.tile(
                        [P, 2 * E], f32, name=f"ops{m}_{pr}", tag="ops"
                    )
                    for half in range(2):
                        tp = pr * 2 + half
                        q, r = tp // 4, tp % 4
                        nc.tensor.matmul(
                            out=ops[:, half * E : (half + 1) * E],
                            lhsT=sT[32 * r : 32 * r + N, q * P : (q + 1) * P],
                            rhs=wt_bf[32 * r : 32 * r + N, :],
                            start=True,
                            stop=True,
                            tile_position=(32 * r, 0),
                        )
                    # copies are deprioritized: the list scheduler slots them
                    # into ACT/DVE idle time behind the chain ops
                    old_prio = tc.cur_priority
                    tc.cur_priority = old_prio + COPY_LAG
                    dst = outsb[:, pr * 2 * E : (pr + 1) * 2 * E]
                    ceng = nc.scalar.copy if pr < act_pairs else (
                        lambda out, in_: nc.vector.tensor_copy(out=out, in_=in_)
                    )
                    if m < SPLIT_COPY_MACROS:
                        # latency-critical fill: copy+DMA per single tp row
                        for half in range(2):
                            tp = pr * 2 + half
                            ceng(
                                out=outsb[:, tp * E : (tp + 1) * E],
                                in_=ops[:, half * E : (half + 1) * E],
                            )
                            nc.sync.dma_start(
                                out=out3[:, g0 + tp : g0 + tp + 1, :],
                                in_=outsb3[:, tp : tp + 1, :],
                            )
                    else:
                        ceng(out=dst, in_=ops[:])
                        if m < FINE_DMA_COUNT:
                            nc.sync.dma_start(
                                out=out3[:, g0 + 2 * pr : g0 + 2 * pr + 2, :],
                                in_=outsb3[:, 2 * pr : 2 * pr + 2, :],
                            )
                        elif pr == act_pairs - 1:
                            nc.sync.dma_start(
                                out=out3[:, g0 : g0 + 2 * act_pairs, :],
                                in_=outsb3[:, 0 : 2 * act_pairs, :],
                            )
                        elif pr == npairs - 1:
                            nc.sync.dma_start(
                                out=out3[:, g0 + 2 * act_pairs : g0 + ms, :],
                                in_=outsb3[:, 2 * act_pairs : ms, :],
                            )
                    tc.cur_priority = old_prio

            warm_count = [0]

            def emit_warmup(n):
                for _ in range(n):
                    psw = pst_pool.tile(
                        [P, P], bf16, name=f"psw{warm_count[0]}", tag="psT"
                    )
                    warm_count[0] += 1
                    nc.tensor.transpose(out=psw[:, 0:P], in_=id_bf, identity=id_bf)

            # pairwise-interleaved emission over the macro list
            nmac = len(MSIZES)
            ctx = {k: {} for k in ("outsb", "theta", "th3", "diff", "sdiff",
                                   "spad", "sT")}
            solo = SOLO_COUNT
            groups = [(m,) for m in range(solo)] + [
                tuple(m for m in (mm, mm + 1) if m < nmac)
                for mm in range(solo, nmac, 2)
            ]
            for gi, pair_ms in enumerate(groups):
                if gi < WEAVE_GROUPS:
                    emit_warmup(WEAVE_N)
                for m in pair_ms:
                    emit_init(ctx, m)
                for r in range(3):
                    for m in pair_ms:
                        emit_sub(ctx, m, r)
                    for m in pair_ms:
                        emit_sin(ctx, m, r)
                    for m in pair_ms:
                        emit_mulredadd(ctx, m, r)
                for m in pair_ms:
                    emit_fsin(ctx, m)
                for m in pair_ms:
                    emit_transpose(ctx, m)
                for m in pair_ms:
                    emit_matmuls(ctx, m)
    return nc


def prep_inputs(phase, omega, K, W):
    """Host-side (numpy) prep: shard phase, replicate tiny params into the
    consolidated per-core constant block."""
    phase = np.ascontiguousarray(np.asarray(phase, dtype=np.float32))
    omega = np.asarray(omega, dtype=np.float32)
    K = np.asarray(K, dtype=np.float32)
    W = np.asarray(W, dtype=np.float32)

    wtrep = np.zeros((P, E), dtype=np.float32)
    wt = np.ascontiguousarray(W.T)  # (6, 512)
    for r in range(4):
        wtrep[32 * r : 32 * r + N, :] = wt
    krep = np.broadcast_to((0.1 * K).reshape(1, NN), (P, NN))
    omrep = np.broadcast_to(omega.reshape(1, N), (P, N))
    identity = np.eye(P, dtype=np.float32)

    in_maps = []
    for c in range(N_CORES):
        cin = np.concatenate(
            [
                phase[c * BC : (c + 1) * BC].reshape(P, G),
                wtrep,
                krep,
                omrep,
                identity,
            ],
            axis=1,
        ).astype(np.float32)
        in_maps.append({"cin": np.ascontiguousarray(cin)})
    return in_maps


def _split_multiwaits(nc):
    """This walrus build rejects any instruction with >1 sem wait. Split:
    move extra waits onto sequencer-level NOPs inserted just before the
    instruction on the same engine queue (in-order dispatch => identical
    semantics)."""
    import concourse.mybir as mybir

    n_split = 0
    for f in nc.m.functions:
        for bb in f.blocks:
            new = []
            for inst in bb.instructions:
                si = inst.sync_info
                waits = list(si.on_wait) if (si is not None and si.on_wait) else []
                if len(waits) > 1:
                    for w in waits[:-1]:
                        nop = mybir.InstNoOp(
                            name=f"WSPLIT-{n_split}", ins=[], outs=[]
                        )
                        n_split += 1
                        nop.engine = inst.engine
                        nop.sync_info = mybir.SyncInfo(on_wait=[w], on_update=[])
                        new.append(nop)
                    inst.sync_info = mybir.SyncInfo(
                        on_wait=[waits[-1]], on_update=list(si.on_update or [])
                    )
                new.append(inst)
            bb.instructions = new
    return n_split


def run(in_maps, trace=False):
    from concourse.bass_utils import run_bass_kernel_spmd

    nc = build_bass()
    _split_multiwaits(nc)
    res = run_bass_kernel_spmd(
        nc, in_maps, core_ids=list(range(N_CORES)), trace=trace
    )
    out = np.concatenate([r["out"] for r in res.results], axis=0)
    return out, res


def kernel(phase, omega, K, W):
    in_maps = prep_inputs(phase, omega, K, W)
    out, _ = run(in_maps, trace=os.environ.get("KURAMOTO_TRACE", "") == "1")
    return out


# revision 6
# speedup vs baseline: 1.0040x; 1.0040x over previous
"""Trainium2 Bass kernel for nn_KuramotoChamber (Kuramoto oscillator chamber).

reference:
    theta = phase[:, None] * omega[None, :]           # (B, 6)
    3x:  dtheta_i = sum_j K[i,j]*sin(theta_j - theta_i); theta += 0.1*dtheta
    out = sin(theta) @ W.T                            # (B, 512)

B = 262144, output is 512 MB fp32 -> memory (output-write) bound; the
per-core roofline is the 64 MiB output write at ~360 GB/s = ~187 us.
Sharding: pure data parallel over the batch across 8 cores.

Final design (TimelineSim ~211 us/core vs 269 us baseline; HW-verified
rel err 2.3e-3):
  - matmuls in bf16 (1 PE cycle/row vs 4 for fp32); bf16 spad/transposes.
  - PSUM->SBUF output copies on ACT (pairs < half) and Pool (rest), off
    the DVE chain; ACT sins never queue behind bulk copies.
  - one output DMA per copy engine per macro (single-wait DMAs).
  - macros emitted pairwise-interleaved (2-deep software pipeline) to
    hide ACT sin latency inside the DVE chain.
  - macro sizes ramp 4,4,8,16... so the first output DMA fires after a
    ~9 us mini-chain instead of a 24 us full-macro chain (DMA engines
    are gap-free in steady state; the fill was the remaining loss).

Per-core dataflow (BC = 32768 batch rows = 128 partitions x 256 groups):
  - consolidated const DMA: [phase (128,256) | wtrep f32 | 0.1K | omega |
    identity f32]; wtrep/identity converted to bf16 on-chip once.
  - chain per macro (DVE): theta init; 3x [diff sub, ACT Sin, *K mul,
    segment-reduce, add]; final sin -> bf16 spad; PE transposes -> psT;
    DVE copy -> sT; matmuls (K=6 strips at partitions {0,32,64,96},
    N=512) -> PSUM f32; ACT/Pool copies -> outsb f32; DMAs out.
"""

import os

import numpy as np

B = 262144
N_CORES = 8
BC = B // N_CORES  # 32768 per core
E = 512
N = 6
NN = N * N  # 36
P = 128
G = BC // P  # 256 groups per core

# macro-size ramp (groups per macro); sum must be G
MSIZES = [4] + [8] * 10 + [12] + [16] * 10
assert sum(MSIZES) == G
SOLO_COUNT = 1  # first macros emitted un-interleaved (latency over throughput)
PE_WARMUP = 0   # dummy transposes before first matmul (PE p-state ramp)
FINE_DMA_COUNT = 9  # macros using one DMA per pair (fill phase)
WEAVE_GROUPS = 0  # emission groups prefixed with PE warmup transposes
WEAVE_N = 0
SUB_ON_POOL = False  # diff-sub on GPSIMD instead of DVE
MUL_ON_POOL = True   # K-mult on GPSIMD instead of DVE
RED_ON_POOL = False  # segment-reduce on GPSIMD instead of DVE
ACT_COPY_PAIRS = 6   # pairs copied by ACT; the rest by DVE (Pool cannot
                     # read PSUM on this hardware)
COPY_PRIO = 1_000_000  # (unused) legacy absolute copy priority
COPY_LAG = 80  # copies scheduled as-if emitted this many instructions later
FILL_MACROS = 2   # macros before steady-state engine assignment kicks in
FILL_ACT_PAIRS = 6  # copy split during fill (DVE has slack there)
CHAIN_WARM_MACROS = 0  # macros whose init is followed by a PE warmup burst
SPLIT_COPY_MACROS = 0  # macros with per-single-tp copies+DMAs (fill latency)
SUB_POOL_MACROS = ()   # specific macros whose diff-sub runs on GPSIMD
MUL_DVE_MACROS = (3, 5, 7)  # fill macros whose K-mult stays on DVE
CHAIN_WARM_N = 32      # transposes per burst (keeps PE p-state hot to MM)
WORK_BUFS = 6
BIG_BUFS = 6
OUTSB_BUFS = 4

# consolidated const layout: [phase(256) | wt(512) | krep(36) | om(6) | id(128)]
OFF_PHASE = 0
OFF_WT = OFF_PHASE + G
OFF_K = OFF_WT + E
OFF_OM = OFF_K + NN
OFF_ID = OFF_OM + N
CIN_W = OFF_ID + P  # 938


def build_bass():
    import concourse.bass as bass
    import concourse.mybir as mybir
    import concourse.tile as tile

    f32 = mybir.dt.float32
    bf16 = mybir.dt.bfloat16
    Sin = mybir.ActivationFunctionType.Sin
    sub_op = mybir.AluOpType.subtract
    mult_op = mybir.AluOpType.mult

    nc = bass.Bass()
    cin = nc.dram_tensor("cin", [P, CIN_W], f32, kind="ExternalInput")
    out = nc.dram_tensor("out", [BC, E], f32, kind="ExternalOutput")

    # per-macro start group offsets
    starts = []
    s = 0
    for ms in MSIZES:
        starts.append(s)
        s += ms

    with tile.TileContext(nc) as tc:
        with (
            tc.tile_pool(name="consts", bufs=1) as consts,
            tc.tile_pool(name="work", bufs=WORK_BUFS) as work,
            tc.tile_pool(name="big", bufs=BIG_BUFS) as big,
            tc.tile_pool(name="outsb", bufs=OUTSB_BUFS) as outsb_pool,
            tc.tile_pool(name="pst", bufs=2, space="PSUM") as pst_pool,
            tc.tile_pool(name="outps", bufs=3, space="PSUM") as outps_pool,
        ):
            cin_sb = consts.tile([P, CIN_W], f32)
            # chain-critical constants first (phase/K/omega), then wt/id;
            # the chain's first op waits only on the first, smaller DMA.
            nc.sync.dma_start(out=cin_sb[:, OFF_PHASE:OFF_WT], in_=cin[:, OFF_PHASE:OFF_WT])
            nc.sync.dma_start(out=cin_sb[:, OFF_K:CIN_W], in_=cin[:, OFF_K:CIN_W])
            nc.sync.dma_start(out=cin_sb[:, OFF_WT:OFF_K], in_=cin[:, OFF_WT:OFF_K])
            phase_sb = cin_sb[:, OFF_PHASE:OFF_WT]
            wt_sb = cin_sb[:, OFF_WT:OFF_K]
            krep_sb = cin_sb[:, OFF_K:OFF_OM]
            om_sb = cin_sb[:, OFF_OM:OFF_ID]
            id_sb = cin_sb[:, OFF_ID:CIN_W]

            # One-time bf16 conversions (each also serves as that engine's
            # first-touch wait on the const DMA).
            wt_bf = consts.tile([P, E], bf16)
            nc.scalar.copy(out=wt_bf, in_=wt_sb)
            id_bf = consts.tile([P, P], bf16)
            nc.gpsimd.tensor_copy(out=id_bf, in_=id_sb)

            # Persistent bf16 sin staging tiles (ping-pong across macros);
            # memset once so pad columns flowing through the transpose hold
            # defined values.
            spads = [
                consts.tile([P, 4 * P], bf16, name=f"spad{i}", tag=f"spad{i}")
                for i in range(2)
            ]
            for sp in spads:
                nc.vector.memset(sp, 0.0)

            # PE p-state warmup: a stream of dummy transposes ramps the PE
            # clock before the first real matmuls (cold-state matmuls cost
            # 2-4x); results are discarded (psT ring slots get overwritten).
            for _ in range(PE_WARMUP):
                psw = pst_pool.tile([P, P], bf16, name="psw", tag="psT")
                nc.tensor.transpose(out=psw[:, 0:P], in_=id_bf, identity=id_bf)

            # DRAM view: row b = p*256 + g  (g = absolute group index)
            out3 = out[:, :].rearrange("(p g) e -> p g e", p=P, g=G)

            kview = krep_sb.rearrange("p (i j) -> p i j", j=N).unsqueeze(1)
            omview = om_sb.unsqueeze(1)

            def emit_init(ctx, m):
                ms = MSIZES[m]
                ctx["outsb"][m] = outsb_pool.tile(
                    [P, 16 * E], f32, name=f"outsb{m}", tag="outsb"
                )
                ctx["theta"][m] = work.tile(
                    [P, 16 * N], f32, name=f"theta{m}", tag="theta"
                )
                th3 = ctx["theta"][m][:, : ms * N].rearrange(
                    "p (t n) -> p t n", n=N
                )
                ctx["th3"][m] = th3
                g0 = starts[m]
                ph = phase_sb[:, g0 : g0 + ms]
                nc.vector.tensor_tensor(
                    out=th3,
                    in0=ph.unsqueeze(2).broadcast_to([P, ms, N]),
                    in1=omview.broadcast_to([P, ms, N]),
                    op=mult_op,
                )
                if m < CHAIN_WARM_MACROS:
                    emit_warmup(CHAIN_WARM_N)

            def emit_sub(ctx, m, r):
                ms = MSIZES[m]
                th3 = ctx["th3"][m]
                diff = big.tile([P, 16 * NN], f32, name=f"diff{m}", tag="diff")
                ctx["diff"][m] = diff
                sub_pool = SUB_ON_POOL or (m in SUB_POOL_MACROS)
                eng = nc.gpsimd if sub_pool else nc.vector
                eng.tensor_tensor(
                    out=diff[:, : ms * NN].rearrange(
                        "p (t i j) -> p t i j", i=N, j=N
                    ),
                    in0=th3.unsqueeze(2).broadcast_to([P, ms, N, N]),
                    in1=th3.unsqueeze(3).broadcast_to([P, ms, N, N]),
                    op=sub_op,
                )

            def emit_sin(ctx, m, r):
                ms = MSIZES[m]
                sdiff = big.tile([P, 16 * NN], f32, name=f"sdiff{m}", tag="sdiff")
                ctx["sdiff"][m] = sdiff
                nc.scalar.activation(
                    out=sdiff[:, : ms * NN],
                    in_=ctx["diff"][m][:, : ms * NN],
                    func=Sin,
                )

            def emit_mulredadd(ctx, m, r):
                ms = MSIZES[m]
                prod = big.tile([P, 16 * NN], f32, name=f"prod{m}", tag="prod")
                mul_pool = (MUL_ON_POOL and m >= FILL_MACROS
                            and m not in MUL_DVE_MACROS)
                meng = nc.gpsimd if mul_pool else nc.vector
                meng.tensor_tensor(
                    out=prod[:, : ms * NN].rearrange(
                        "p (t i j) -> p t i j", i=N, j=N
                    ),
                    in0=ctx["sdiff"][m][:, : ms * NN].rearrange(
                        "p (t i j) -> p t i j", i=N, j=N
                    ),
                    in1=kview.broadcast_to([P, ms, N, N]),
                    op=mult_op,
                )
                dth = work.tile([P, 16 * N], f32, name=f"dth{m}", tag="dth")
                reng = nc.gpsimd if RED_ON_POOL else nc.vector
                reng.reduce_sum(
                    out=dth[:, : ms * N],
                    in_=prod[:, : ms * NN].rearrange("p (ti j) -> p ti j", j=N),
                    axis=mybir.AxisListType.X,
                )
                nc.vector.tensor_add(
                    out=ctx["theta"][m][:, : ms * N],
                    in0=ctx["theta"][m][:, : ms * N],
                    in1=dth[:, : ms * N],
                )

            def emit_fsin(ctx, m):
                ms = MSIZES[m]
                qb = (ms + 3) // 4  # q-blocks of up to 4 groups
                rdim = min(ms, 4)
                spad = spads[m % 2]
                ctx["spad"][m] = spad
                # group t = 4q+r at cols [128q+32r, +6)
                sp4 = spad[:, : qb * P].rearrange(
                    "p (q r c) -> p q r c", q=qb, r=rdim
                )
                nc.scalar.activation(
                    out=sp4[:, :, :, 0:N],
                    in_=ctx["theta"][m][:, : ms * N].rearrange(
                        "p (q r n) -> p q r n", q=qb, r=rdim
                    ),
                    func=Sin,
                )

            def emit_transpose(ctx, m):
                ms = MSIZES[m]
                qb = (ms + 3) // 4
                spad = ctx["spad"][m]
                psT = pst_pool.tile([P, 4 * P], bf16, name=f"psT{m}", tag="psT")
                for q in range(qb):
                    nc.tensor.transpose(
                        out=psT[:, q * P : (q + 1) * P],
                        in_=spad[:, q * P : (q + 1) * P],
                        identity=id_bf,
                    )
                sT = work.tile([P, 4 * P], bf16, name=f"sT{m}", tag="sT")
                ctx["sT"][m] = sT
                nc.vector.tensor_copy(
                    out=sT[:, : qb * P], in_=psT[:, : qb * P]
                )

            def emit_matmuls(ctx, m):
                ms = MSIZES[m]
                sT = ctx["sT"][m]
                outsb = ctx["outsb"][m]
                npairs = ms // 2
                acp = ACT_COPY_PAIRS if m >= FILL_MACROS else FILL_ACT_PAIRS
                act_pairs = max(1, (npairs * acp) // 8)
                g0 = starts[m]
                outsb3 = outsb[:, : ms * E].rearrange(
                    "p (t e) -> p t e", t=ms
                )
                for pr in range(npairs):
                    ops = outps_pool.tile(
                        [P, 2 * E], f32, name=f"ops{m}_{pr}", tag="ops"
                    )
                    for half in range(2):
                        tp = pr * 2 + half
                        q, r = tp // 4, tp % 4
                        nc.tensor.matmul(
                            out=ops[:, half * E : (half + 1) * E],
                            lhsT=sT[32 * r : 32 * r + N, q * P : (q + 1) * P],
                            rhs=wt_bf[32 * r : 32 * r + N, :],
                            start=True,
                            stop=True,
                            tile_position=(32 * r, 0),
                        )
                    # copies are deprioritized: the list scheduler slots them
                    # into ACT/DVE idle time behind the chain ops
                    old_prio = tc.cur_priority
                    tc.cur_priority = old_prio + COPY_LAG
                    dst = outsb[:, pr * 2 * E : (pr + 1) * 2 * E]
                    ceng = nc.scalar.copy if pr < act_pairs else (
                        lambda out, in_: nc.vector.tensor_copy(out=out, in_=in_)
                    )
                    if m < SPLIT_COPY_MACROS:
                        # latency-critical fill: copy+DMA per single tp row
                        for half in range(2):
                            tp = pr * 2 + half
                            ceng(
                                out=outsb[:, tp * E : (tp + 1) * E],
                                in_=ops[:, half * E : (half + 1) * E],
                            )
                            nc.sync.dma_start(
                                out=out3[:, g0 + tp : g0 + tp + 1, :],
                                in_=outsb3[:, tp : tp + 1, :],
                            )
                    else:
                        ceng(out=dst, in_=ops[:])
                        if m < FINE_DMA_COUNT:
                            nc.sync.dma_start(
                                out=out3[:, g0 + 2 * pr : g0 + 2 * pr + 2, :],
                                in_=outsb3[:, 2 * pr : 2 * pr + 2, :],
                            )
                        elif pr == act_pairs - 1:
                            nc.sync.dma_start(
                                out=out3[:, g0 : g0 + 2 * act_pairs, :],
                                in_=outsb3[:, 0 : 2 * act_pairs, :],
                            )
                        elif pr == npairs - 1:
                            nc.sync.dma_start(
                                out=out3[:, g0 + 2 * act_pairs : g0 + ms, :],
                                in_=outsb3[:, 2 * act_pairs : ms, :],
                            )
                    tc.cur_priority = old_prio

            warm_count = [0]

            def emit_warmup(n):
                for _ in range(n):
                    psw = pst_pool.tile(
                        [P, P], bf16, name=f"psw{warm_count[0]}", tag="psT"
                    )
                    warm_count[0] += 1
                    nc.tensor.transpose(out=psw[:, 0:P], in_=id_bf, identity=id_bf)

            # pairwise-interleaved emission over the macro list
            nmac = len(MSIZES)
            ctx = {k: {} for k in ("outsb", "theta", "th3", "diff", "sdiff",
                                   "spad", "sT")}
            solo = SOLO_COUNT
            groups = [(m,) for m in range(solo)] + [
                tuple(m for m in (mm, mm + 1) if m < nmac)
                for mm in range(solo, nmac, 2)
            ]
            for gi, pair_ms in enumerate(groups):
                if gi < WEAVE_GROUPS:
                    emit_warmup(WEAVE_N)
                for m in pair_ms:
                    emit_init(ctx, m)
                for r in range(3):
                    for m in pair_ms:
                        emit_sub(ctx, m, r)
                    for m in pair_ms:
                        emit_sin(ctx, m, r)
                    for m in pair_ms:
                        emit_mulredadd(ctx, m, r)
                for m in pair_ms:
                    emit_fsin(ctx, m)
                for m in pair_ms:
                    emit_transpose(ctx, m)
                for m in pair_ms:
                    emit_matmuls(ctx, m)
    return nc


def prep_inputs(phase, omega, K, W):
    """Host-side (numpy) prep: shard phase, replicate tiny params into the
    consolidated per-core constant block."""
    phase = np.ascontiguousarray(np.asarray(phase, dtype=np.float32))
    omega = np.asarray(omega, dtype=np.float32)
    K = np.asarray(K, dtype=np.float32)
    W = np.asarray(W, dtype=np.float32)

    wtrep = np.zeros((P, E), dtype=np.float32)
    wt = np.ascontiguousarray(W.T)  # (6, 512)
    for r in range(4):
        wtrep[32 * r : 32 * r + N, :] = wt
    krep = np.broadcast_to((0.1 * K).reshape(1, NN), (P, NN))
    omrep = np.broadcast_to(omega.reshape(1, N), (P, N))
    identity = np.eye(P, dtype=np.float32)

    in_maps = []
    for c in range(N_CORES):
        cin = np.concatenate(
            [
                phase[c * BC : (c + 1) * BC].reshape(P, G),
                wtrep,
                krep,
                omrep,
                identity,
            ],
            axis=1,
        ).astype(np.float32)
        in_maps.append({"cin": np.ascontiguousarray(cin)})
    return in_maps


def _split_multiwaits(nc):
    """This walrus build rejects any instruction with >1 sem wait. Split:
    move extra waits onto sequencer-level NOPs inserted just before the
    instruction on the same engine queue (in-order dispatch => identical
    semantics)."""
    import concourse.mybir as mybir

    n_split = 0
    for f in nc.m.functions:
        for bb in f.blocks:
            new = []
            for inst in bb.instructions:
                si = inst.sync_info
                waits = list(si.on_wait) if (si is not None and si.on_wait) else []
                if len(waits) > 1:
                    for w in waits[:-1]:
                        nop = mybir.InstNoOp(
                            name=f"WSPLIT-{n_split}", ins=[], outs=[]
                        )
                        n_split += 1
                        nop.engine = inst.engine
                        nop.sync_info = mybir.SyncInfo(on_wait=[w], on_update=[])
                        new.append(nop)
                    inst.sync_info = mybir.SyncInfo(
                        on_wait=[waits[-1]], on_update=list(si.on_update or [])
                    )
                new.append(inst)
            bb.instructions = new
    return n_split


def run(in_maps, trace=False):
    from concourse.bass_utils import run_bass_kernel_spmd

    nc = build_bass()
    _split_multiwaits(nc)
    res = run_bass_kernel_spmd(
        nc, in_maps, core_ids=list(range(N_CORES)), trace=trace
    )
    out = np.concatenate([r["out"] for r in res.results], axis=0)
    return out, res


def kernel(phase, omega, K, W):
    in_maps = prep_inputs(phase, omega, K, W)
    out, _ = run(in_maps, trace=os.environ.get("KURAMOTO_TRACE", "") == "1")
    return out
